# revision 4
# baseline (speedup 1.0000x reference)
"""Bilinear interpolation (affine grid sampling) Trainium2 Bass kernel.

Problem: image [32,256,256,32] f32, theta [32,6] f32 -> out [32,256,256,32] f32.
Sharding: pure data parallel over batch; 4 samples per core on 8 cores.

Strategy per core:
  - Host builds a "row-pair image": unit(b,y,x) = [I[b,y,x,:], I[b,min(y+1,255),x,:]]
    (64 f32 = 256B per unit). One 512B gather at unit index (b*65536 + y0*256 + x0)
    fetches all 4 bilinear corners contiguously at full DMA descriptor rate.
  - On device: affine coord generation, truncating casts, clipped corner indices,
    bilinear weights with clip-folding (x-adjacency and bottom-row folds), one
    indirect DMA gather per 4096-pixel batch, weighted blend on DVE, contiguous
    stores.
"""

import sys

sys.path.insert(0, "/opt/trn_rl_repo")

from contextlib import ExitStack

import numpy as np

import concourse.bacc as bacc
import concourse.bass as bass
import concourse.tile as tile
from concourse import mybir
from concourse.bass_utils import run_bass_kernel_spmd

# Problem geometry (hardcoded per task contract)
B_TOTAL = 32
N_CORES = 8
S = B_TOTAL // N_CORES          # samples per core
H = W = 256
C = 32
HW = H * W                      # 65536 pixels per sample
UNITS = S * HW                  # gather units per core (1 unit = 64 f32 = 256B)
PAD_UNITS = 16                  # padding so the last 512B gather stays in-bounds
P = 128                         # SBUF partitions
KB = 32                         # pixels per partition per gather batch
NB = HW // (P * KB)             # 16 gather batches per sample
Q = NB * KB                     # 512 free columns in the per-sample coord tiles

_COMPILED = {}


def _build_nc():
    f32 = mybir.dt.float32
    i32 = mybir.dt.int32
    nc = bacc.Bacc("TRN2", target_bir_lowering=False)

    tbl = nc.dram_tensor("tbl", [UNITS + PAD_UNITS, 64], f32, kind="ExternalInput")
    xg_d = nc.dram_tensor("xg", [P, Q], f32, kind="ExternalInput")
    yg_d = nc.dram_tensor("yg", [P, Q], f32, kind="ExternalInput")
    th_d = nc.dram_tensor("th", [P, 6 * S], f32, kind="ExternalInput")
    out_d = nc.dram_tensor("out", [S, NB, P, KB, C], f32, kind="ExternalOutput")

    AF = mybir.AluOpType
    V = nc.vector

    with tile.TileContext(nc) as tc, ExitStack() as ctx:
        singles = ctx.enter_context(tc.tile_pool(name="singles", bufs=1))
        coord = ctx.enter_context(tc.tile_pool(name="coord", bufs=1))
        persist = ctx.enter_context(tc.tile_pool(name="persist", bufs=2))
        gpool = ctx.enter_context(tc.tile_pool(name="gpool", bufs=2))
        opool = ctx.enter_context(tc.tile_pool(name="opool", bufs=2))

        xg = singles.tile([P, Q], f32)
        yg = singles.tile([P, Q], f32)
        th = singles.tile([P, 6 * S], f32)
        nc.sync.dma_start(out=xg[:], in_=xg_d[:])
        nc.sync.dma_start(out=yg[:], in_=yg_d[:])
        nc.sync.dma_start(out=th[:], in_=th_d[:])

        def ctile(tag, dt=f32):
            return coord.tile([P, Q], dt, tag=tag, name=tag)

        for b in range(S):
            t = [th[:, 6 * b + k : 6 * b + k + 1] for k in range(6)]

            def affine(t0, t1, t2, tag):
                # x = ((t0*xg + t1*yg) + t2 + 1) * 128, elementwise f32
                a = ctile("scrA")
                bb = ctile("scrB")
                V.tensor_scalar(out=a[:], in0=xg[:], scalar1=t0,
                                scalar2=None, op0=AF.mult)
                V.tensor_scalar(out=bb[:], in0=yg[:], scalar1=t1,
                                scalar2=None, op0=AF.mult)
                v = ctile(tag)
                V.tensor_tensor(out=v[:], in0=a[:], in1=bb[:], op=AF.add)
                V.tensor_scalar(out=v[:], in0=v[:], scalar1=t2,
                                scalar2=None, op0=AF.add)
                V.tensor_scalar(out=v[:], in0=v[:], scalar1=1.0,
                                scalar2=128.0, op0=AF.add, op1=AF.mult)
                return v

            x = affine(t[0], t[1], t[2], "x")
            y = affine(t[3], t[4], t[5], "y")

            def trunc_f(v, tag):
                # HW f32->i32 cast rounds to nearest-even; fix up to truncation
                # toward zero: for v>=0 subtract 1 where rounded f > v (floor),
                # for v<0 add 1 where rounded f < v (ceil).
                vi = ctile("scrI", i32)
                V.tensor_copy(out=vi[:], in_=v[:])
                vf = ctile(tag)
                V.tensor_copy(out=vf[:], in_=vi[:])
                gt = ctile("scrC")
                V.tensor_tensor(out=gt[:], in0=vf[:], in1=v[:], op=AF.is_gt)
                lt = ctile("scrD")
                V.tensor_tensor(out=lt[:], in0=vf[:], in1=v[:], op=AF.is_lt)
                nn = ctile("scrE")
                V.tensor_scalar(out=nn[:], in0=v[:], scalar1=0.0,
                                scalar2=None, op0=AF.is_ge)
                ng = ctile("scrF")
                V.tensor_scalar(out=ng[:], in0=v[:], scalar1=0.0,
                                scalar2=None, op0=AF.is_lt)
                V.tensor_tensor(out=gt[:], in0=gt[:], in1=nn[:], op=AF.mult)
                V.tensor_tensor(out=lt[:], in0=lt[:], in1=ng[:], op=AF.mult)
                V.tensor_tensor(out=vf[:], in0=vf[:], in1=gt[:], op=AF.subtract)
                V.tensor_tensor(out=vf[:], in0=vf[:], in1=lt[:], op=AF.add)
                return vf

            x0f = trunc_f(x, "x0f")
            y0f = trunc_f(y, "y0f")

            def clip01(v, tag):
                o = ctile(tag)
                V.tensor_scalar(out=o[:], in0=v[:], scalar1=float(W - 1),
                                scalar2=0.0, op0=AF.min, op1=AF.max)
                return o

            x0c = clip01(x0f, "x0c")
            y0c = clip01(y0f, "y0c")

            def clip_plus1(v, tag):
                o = ctile(tag)
                V.tensor_scalar(out=o[:], in0=v[:], scalar1=1.0,
                                scalar2=float(W - 1), op0=AF.add, op1=AF.min)
                V.tensor_scalar(out=o[:], in0=o[:], scalar1=0.0,
                                scalar2=None, op0=AF.max)
                return o

            x1c = clip_plus1(x0f, "x1c")
            y1c = clip_plus1(y0f, "y1c")

            def tt_new(i0, i1, op, tag, pool=None):
                o = ctile(tag) if pool is None else pool.tile([P, Q], f32, tag=tag, name=tag)
                V.tensor_tensor(out=o[:], in0=i0[:], in1=i1[:], op=op)
                return o

            def tt_ip(dst, i1, op):
                V.tensor_tensor(out=dst[:], in0=dst[:], in1=i1[:], op=op)

            u1 = tt_new(x1c, x, AF.subtract, "u1")
            u0 = tt_new(x, x0c, AF.subtract, "u0")
            v1 = tt_new(y1c, y, AF.subtract, "v1")
            v0 = tt_new(y, y0c, AF.subtract, "v0")

            wa = tt_new(u1, v1, AF.mult, "wa", persist)
            wb = tt_new(u1, v0, AF.mult, "wb", persist)
            wc = tt_new(u0, v1, AF.mult, "wc", persist)
            wd = tt_new(u0, v0, AF.mult, "wd", persist)

            cx = tt_new(x1c, x0c, AF.is_equal, "cx")
            cy = tt_new(y1c, y0c, AF.is_equal, "cy")
            sx = ctile("scrA")
            V.tensor_scalar(out=sx[:], in0=cx[:], scalar1=-1.0,
                            scalar2=1.0, op0=AF.mult, op1=AF.add)
            sy = ctile("scrB")
            V.tensor_scalar(out=sy[:], in0=cy[:], scalar1=-1.0,
                            scalar2=1.0, op0=AF.mult, op1=AF.add)

            # bottom-row fold: when clip(y0+1)==y0c the fetched bottom row may be
            # wrong (y<0 case) or equal the top row (y>=255 case, harmless):
            # move the bottom weights onto the top pixels.
            ft = tt_new(wb, cy, AF.mult, "ft")
            tt_ip(wa, ft, AF.add)
            tt_ip(wb, sy, AF.mult)
            ft = tt_new(wd, cy, AF.mult, "ft")
            tt_ip(wc, ft, AF.add)
            tt_ip(wd, sy, AF.mult)

            # x-adjacency fold: when clip(x0+1)==x0c the second fetched unit
            # holds the wrong column: move right-column weights onto the left.
            ft = tt_new(wc, cx, AF.mult, "ft")
            tt_ip(wa, ft, AF.add)
            tt_ip(wc, sx, AF.mult)
            ft = tt_new(wd, cx, AF.mult, "ft")
            tt_ip(wb, ft, AF.add)
            tt_ip(wd, sx, AF.mult)

            idxf = ctile("u1")  # u1 is dead now; reuse its slot
            V.tensor_scalar(out=idxf[:], in0=y0c[:], scalar1=float(W),
                            scalar2=float(b * HW), op0=AF.mult, op1=AF.add)
            tt_ip(idxf, x0c, AF.add)
            idx = persist.tile([P, Q], i32, tag="idx", name="idx")
            V.tensor_copy(out=idx[:], in_=idxf[:])

            for bi in range(NB):
                sl = slice(bi * KB, (bi + 1) * KB)
                gt = gpool.tile([P, KB, 128], f32, tag="gt", name="gt")
                nc.gpsimd.indirect_dma_start(
                    out=gt[:],
                    out_offset=None,
                    in_=tbl[:],
                    in_offset=bass.IndirectOffsetOnAxis(ap=idx[:, sl], axis=0),
                )
                pa = gt[:, :, 0:32]
                pb = gt[:, :, 32:64]
                pc = gt[:, :, 64:96]
                pd = gt[:, :, 96:128]

                def wbc(w):
                    return w[:, sl, None].to_broadcast([P, KB, C])

                ot = opool.tile([P, KB, C], f32, tag="ot", name="ot")
                tm = opool.tile([P, KB, C], f32, tag="tm", name="tm")
                V.tensor_tensor(out=ot[:], in0=pa, in1=wbc(wa), op=AF.mult)
                V.tensor_tensor(out=tm[:], in0=pb, in1=wbc(wb), op=AF.mult)
                V.tensor_tensor(out=ot[:], in0=ot[:], in1=tm[:], op=AF.add)
                V.tensor_tensor(out=tm[:], in0=pc, in1=wbc(wc), op=AF.mult)
                V.tensor_tensor(out=ot[:], in0=ot[:], in1=tm[:], op=AF.add)
                V.tensor_tensor(out=tm[:], in0=pd, in1=wbc(wd), op=AF.mult)
                V.tensor_tensor(out=ot[:], in0=ot[:], in1=tm[:], op=AF.add)

                nc.sync.dma_start(out=out_d[b, bi], in_=ot[:])

    nc.compile()
    return nc


def _host_tables():
    import jax.numpy as jnp

    # bitwise-identical linspace to the reference (computed via jax on host)
    xs = np.asarray(jnp.linspace(-1.0, 1.0, W), dtype=np.float32)
    ys = np.asarray(jnp.linspace(-1.0, 1.0, H), dtype=np.float32)
    p = np.arange(P)[:, None]           # [128,1]
    q = np.arange(Q)[None, :]           # [1,512]
    bi = q // KB
    j = q % KB
    rows_per_batch = (P * KB) // W      # 16 output rows per gather batch
    xg = xs[(KB * p + j) % W].astype(np.float32)                       # [128,512]
    yg = ys[rows_per_batch * bi + p // (P // rows_per_batch)].astype(np.float32)
    return xg, yg


def kernel(image: np.ndarray, theta: np.ndarray) -> np.ndarray:
    image = np.ascontiguousarray(image, dtype=np.float32)
    theta = np.ascontiguousarray(theta, dtype=np.float32)
    assert image.shape == (B_TOTAL, H, W, C) and theta.shape == (B_TOTAL, 6)

    if "nc" not in _COMPILED:
        _COMPILED["nc"] = _build_nc()
    nc = _COMPILED["nc"]

    xg, yg = _host_tables()

    # row-pair image: unit (b,y,x) = [I[b,y,x,:], I[b,min(y+1,255),x,:]]
    ydown = np.concatenate([image[:, 1:], image[:, -1:]], axis=1)   # [32,256,256,32]
    pair = np.concatenate([image, ydown], axis=-1)                   # [32,256,256,64]
    pair = pair.reshape(B_TOTAL * HW, 64)

    in_maps = []
    for c in range(N_CORES):
        tbl_np = np.zeros((UNITS + PAD_UNITS, 64), np.float32)
        tbl_np[:UNITS] = pair[c * UNITS : (c + 1) * UNITS]
        th_np = np.tile(theta[c * S : (c + 1) * S].reshape(1, 6 * S), (P, 1))
        in_maps.append({
            "tbl": tbl_np,
            "xg": xg,
            "yg": yg,
            "th": np.ascontiguousarray(th_np, dtype=np.float32),
        })

    res = run_bass_kernel_spmd(nc, in_maps, core_ids=list(range(N_CORES)))

    out = np.empty((B_TOTAL, H, W, C), np.float32)
    for c in range(N_CORES):
        o = res.results[c]["out"]        # [S, NB, P, KB, C]
        out[c * S : (c + 1) * S] = o.reshape(S, H, W, C)
    return out
